# revision 2
# baseline (speedup 1.0000x reference)
"""Deformable-ROI bilinear feature gather (KeypPointBboxNet) on 8 TRN2 cores.

Strategy (matches the sharding hint): feat_map is sharded on the batch dim
(one image per NeuronCore); rois/offsets are routed host-side to the core
holding their image.

Device-side work is the memory-bound part: per sample point, gather the 4
bilinear-footprint pixels (256 channels each) out of the HBM-resident image
and blend them with the per-point weights.  Everything that is O(N_points)
scalar math (coordinate/floor/weight computation, batch routing) happens on
the host while staging the shards:

  - the image is stored in HBM as fp16 in an interleaved row-pair layout:
    entry (h, w) holds the channels of pixels (h, w) and (h+1, w) back to
    back (1 KB).  One dma_gather element of 2 KB starting at entry
    (h_low, w_low) therefore covers the full 2x2 bilinear footprint with a
    single int16 index per sample point -- half the descriptors of a
    top-row/bottom-row gather pair, at 2 KB per descriptor.
  - per point the host precomputes idx = h_low*W + w_low (int16, gather
    order) and the three blend weights (lh, lw, ch=1-lh) in the [128, S]
    compute layout.
  - on device, per group of CS slots: one SWDGE dma_gather pulls
    [128, CS, 2KB]; ACT applies ch to the top rows, DVE fma's the bottom
    rows and does the horizontal lerp; results are stored group-by-group as
    fp16 (upcast on host), overlapping the next group's gather.

fp16 (not bf16) keeps the quantization error ~5e-4 even under the negative
extrapolation weights the reference's low-edge rule produces, far inside
the 2e-2 gate, while halving both HBM gather traffic and output traffic.
"""

import math

import numpy as np

B, C, H, W = 8, 256, 128, 128
N_ROIS, NUM_POINT, STRIDE = 2048, 9, 8
NCORES = 8
CS = 5  # slots (of 128 points) per dma_gather call
# Interleaved fm entries: idx can reach H*W - 1 and each gather element reads
# entries idx and idx+1 -> one pad entry; round up a little.
FMI_ROWS = H * W + 8

_PROGRAM_CACHE: dict[tuple, object] = {}


def _build_program(S: int, reps: int = 1, cs: int = CS):
    import concourse.bacc as bacc
    import concourse.mybir as mybir
    import concourse.tile as tile
    from concourse.bass_types import AP

    f32 = mybir.dt.float32
    f16 = mybir.dt.float16
    i16 = mybir.dt.int16
    op = mybir.AluOpType
    assert S % cs == 0
    G = S // cs

    nc = bacc.Bacc("TRN2", target_bir_lowering=False, debug=False, num_devices=NCORES)
    fm_t = nc.dram_tensor("fm", [FMI_ROWS, 2 * C], f16, kind="ExternalInput")
    idx_t = nc.dram_tensor("idx", [128, S * 8], i16, kind="ExternalInput")
    wgt_t = nc.dram_tensor("wgt", [128, 3 * S], f32, kind="ExternalInput")
    out_t = nc.dram_tensor("out", [128, S * C], f16, kind="ExternalOutput")

    # fm viewed as overlapping [entry, 4*C] rows with stride 2*C: one gathered
    # element covers interleaved entries (h,w) and (h,w+1) = the 2x2 footprint.
    fm_gather_ap = AP(fm_t, 0, [[2 * C, FMI_ROWS - 1], [1, 4 * C]])

    with tile.TileContext(nc) as tc:
        with (
            tc.tile_pool(name="const", bufs=1) as cpool,
            tc.tile_pool(name="gath", bufs=2) as gpool,
            tc.tile_pool(name="work", bufs=3) as wpool,
            tc.tile_pool(name="outp", bufs=2) as opool,
        ):

            def body():
                idx = cpool.tile([128, S * 8], i16, tag="idx")
                nc.sync.dma_start(idx[:], idx_t[:])
                wgt = cpool.tile([128, 3 * S], f32, tag="wgt")
                nc.sync.dma_start(wgt[:], wgt_t[:])
                for g in range(G):
                    gt = gpool.tile([128, cs, 4 * C], f16, tag="gt")
                    nc.gpsimd.dma_gather(
                        gt[:], fm_gather_ap, idx[:, g * cs * 8 : (g + 1) * cs * 8],
                        cs * 128, cs * 128, 4 * C, elem_step=2 * C,
                    )
                    og = opool.tile([128, cs * C], f16, tag="og")
                    for sl in range(cs):
                        s = g * cs + sl
                        gv = gt[:, sl, :].rearrange(
                            "p (a b c) -> p a b c", a=2, b=2, c=C
                        )
                        t1 = wpool.tile([128, 2, C], f16, tag="t1")
                        nc.scalar.activation(
                            t1[:], gv[:, :, 0, :], mybir.ActivationFunctionType.Copy,
                            bias=0.0, scale=wgt[:, 2 * S + s : 2 * S + s + 1],
                        )
                        st = wpool.tile([128, 2, C], f32, tag="st")
                        nc.vector.scalar_tensor_tensor(
                            st[:], gv[:, :, 1, :], wgt[:, s : s + 1], t1[:],
                            op.mult, op.add,
                        )
                        d = wpool.tile([128, C], f32, tag="d")
                        nc.vector.tensor_tensor(d[:], st[:, 1, :], st[:, 0, :], op.subtract)
                        nc.vector.scalar_tensor_tensor(
                            og[:, sl * C : (sl + 1) * C], d[:],
                            wgt[:, S + s : S + s + 1], st[:, 0, :], op.mult, op.add,
                        )
                    nc.sync.dma_start(out_t[:, g * cs * C : (g + 1) * cs * C], og[:])

            if reps == 1:
                body()
            else:
                with tc.For_i(0, reps, 1):
                    body()

    nc.compile()
    return nc


def _get_program(S: int):
    key = (S, 1, CS)
    if key not in _PROGRAM_CACHE:
        _PROGRAM_CACHE[key] = _build_program(S)
    return _PROGRAM_CACHE[key]


def _point_coords(r, off):
    """Per-point gather index + blend weights, float32 math matching the
    reference's edge rules.  r: [nb,5] rois, off: [nb, 2*NUM_POINT].
    Returns idx int32 [nb*P], lh, lw, ch float32 [nb*P]."""
    r = r.astype(np.float32)
    off = off.astype(np.float32).reshape(-1, NUM_POINT, 2)
    cx = (r[:, 1] + r[:, 3]) * np.float32(0.5)
    cy = (r[:, 2] + r[:, 4]) * np.float32(0.5)
    w_ = r[:, 3] - r[:, 1] + np.float32(1.0)
    h_ = r[:, 4] - r[:, 2] + np.float32(1.0)
    ix = (cx[:, None] + off[:, :, 0] * w_[:, None] * np.float32(0.1)) / np.float32(STRIDE)
    iy = (cy[:, None] + off[:, :, 1] * h_[:, None] * np.float32(0.1)) / np.float32(STRIDE)
    wl = np.clip(np.floor(ix), 0, W - 1)
    lw = np.where(wl >= W - 1, np.float32(0.0), (ix - wl).astype(np.float32))
    hl = np.clip(np.floor(iy), 0, H - 1)
    lh = np.where(hl >= H - 1, np.float32(0.0), (iy - hl).astype(np.float32))
    idx = (hl * W + wl).astype(np.int32).reshape(-1)
    return idx, lh.reshape(-1).astype(np.float32), lw.reshape(-1).astype(np.float32), (
        np.float32(1.0) - lh.reshape(-1)
    ).astype(np.float32)


def _host_prep(feat_map, rois, offset, num_point):
    """Route rois by batch index; build per-core fm/idx/weight inputs."""
    bidx = rois[:, 0].astype(np.int32)
    ids = [np.nonzero(bidx == b)[0] for b in range(B)]
    cap = max(len(i) for i in ids)
    S = math.ceil(max(cap * num_point, 1) / 128)
    S = ((S + CS - 1) // CS) * CS
    NP = S * 128

    in_maps = []
    for b in range(B):
        fmhwc = np.ascontiguousarray(feat_map[b].transpose(1, 2, 0))  # [H,W,C]
        fmi = np.zeros((FMI_ROWS, 2 * C), np.float16)
        fmi[: H * W, :C] = fmhwc.reshape(H * W, C)
        fmi[: (H - 1) * W, C:] = fmhwc[1:].reshape((H - 1) * W, C)
        idxf = np.zeros(NP, np.int16)
        lhf = np.zeros(NP, np.float32)
        lwf = np.zeros(NP, np.float32)
        chf = np.zeros(NP, np.float32)
        idl = ids[b]
        nb = len(idl)
        if nb:
            pidx, lh, lw, ch = _point_coords(rois[idl], offset[idl])
            npts = nb * num_point
            idxf[:npts] = pidx.astype(np.int16)
            lhf[:npts] = lh
            lwf[:npts] = lw
            chf[:npts] = ch
        # gather-order layout: point q=(s*128+p) -> idx col s*8 + p//16,
        # partition p%16, replicated to all 128 partitions.
        idx16 = np.ascontiguousarray(idxf.reshape(S * 8, 16).T)  # [16, S*8]
        idx_dev = np.tile(idx16, (8, 1))  # [128, S*8]
        # weight layout: [128, 3*S] = (lh | lw | ch), point q at [q%128, q//128]
        wgt = np.concatenate(
            [a.reshape(S, 128).T for a in (lhf, lwf, chf)], axis=1
        ).astype(np.float32)
        in_maps.append({"fm": fmi, "idx": idx_dev, "wgt": np.ascontiguousarray(wgt)})
    return ids, S, in_maps


def _host_unshard(results, ids, S, num_point, n):
    out_full = np.zeros((n, num_point, C), np.float32)
    for b in range(B):
        nb = len(ids[b])
        if not nb:
            continue
        o = results[b]["out"].reshape(128, S, C).transpose(1, 0, 2).reshape(S * 128, C)
        out_full[ids[b]] = (
            o[: nb * num_point].astype(np.float32).reshape(nb, num_point, C)
        )
    return out_full


def kernel(feat_map, rois, offset, stride, num_point, _collect=None):
    from concourse.bass_utils import run_bass_kernel_spmd

    feat_map = np.ascontiguousarray(np.asarray(feat_map, np.float32))
    rois = np.asarray(rois, np.float32)
    offset = np.asarray(offset, np.float32)
    stride = int(stride)
    num_point = int(num_point)
    assert feat_map.shape == (B, C, H, W), feat_map.shape
    assert stride == STRIDE and num_point == NUM_POINT

    ids, S, in_maps = _host_prep(feat_map, rois, offset, num_point)
    nc = _get_program(S)
    res = run_bass_kernel_spmd(nc, in_maps, core_ids=list(range(NCORES)),
                               **(_collect.pop("spmd_kwargs", {}) if _collect else {}))
    if _collect is not None:
        _collect["res"] = res
    return _host_unshard(res.results, ids, S, num_point, rois.shape[0])


# revision 5
# speedup vs baseline: 1.1387x; 1.1387x over previous
"""Deformable-ROI bilinear feature gather (KeypPointBboxNet) on 8 TRN2 cores.

Strategy (matches the sharding hint): feat_map is sharded on the batch dim
(one image per NeuronCore); rois/offsets are routed host-side to the core
holding their image.

The device kernel is the memory-bound part: per sample point, gather the 4
bilinear-footprint pixels (256 channels each) out of the HBM-resident image
and blend them with per-point weights.  All O(N_points) scalar bookkeeping
(coordinates, floors, weights, routing) happens on the host while staging
the shards.  Device-side design, driven by measured TRN2 behavior:

  - the image lives in HBM as **int8** (per-image absmax/127 scale, folded
    into the blend weights) in an interleaved row-pair layout: entry (h, w)
    holds the channels of pixels (h, w) and (h+1, w) back to back, so one
    1 KB dma_gather element starting at entry (h_low, w_low) covers the full
    2x2 bilinear footprint with a single int16 index per point.
  - points are sorted by gather index on the host (the inverse permutation
    is applied when unsharding), which makes the SWDGE gather's HBM reads
    near-sequential instead of random.
  - the four gather calls are issued on **four different SWDGE queues**:
    the gather ucode runs each call's descriptor generation on the Q7 core
    pair selected by queue_num, so queues parallelize descgen; same-queue
    calls serialize (~8 us/call measured).  They are also issued back-to-back
    *before* any compute: DVE two-tensor ops hold the shared SBUF port that
    GPSIMD needs for descriptor writes, so descgen must finish while the
    DVE is still idle.
  - padding slots gather entry 0 with all-zero blend weights (trailing
    negative indices would skip descriptors, but the ucode's ring-space
    accounting reserves from num_idxs_reg, which must then carry the
    per-core reduced count -- impossible under SPMD where real counts
    differ per core -- so the safe fixed-shape padding is used).
  - per slot, ACT (own SBUF ports) applies the top-row weight while DVE does
    the bottom-row fma and the horizontal lerp in fp16; each group's 128x5
    fp16 result is stored by HWDGE while later groups still compute.

int8 + fp16 keeps the end-to-end quantization error ~5e-3, far inside the
2e-2 gate, while cutting HBM gather traffic 4x vs the f32 original.
"""

import math

import numpy as np

B, C, H, W = 8, 256, 128, 128
N_ROIS, NUM_POINT, STRIDE = 2048, 9, 8
NCORES = 8
CS = 5  # slots (of 128 points) per dma_gather call
NQ = 4  # SWDGE queues (gather descgen parallelism)
# Interleaved fm entries: idx can reach H*W - 1 and each gather element reads
# entries idx and idx+1 -> one pad entry; round up a little.
FMI_ROWS = H * W + 8

_PROGRAM_CACHE: dict[tuple, object] = {}


def _build_program(S: int, reps: int = 1, cs: int = CS, lerp: str = "dve"):
    import concourse.bacc as bacc
    import concourse.mybir as mybir
    import concourse.tile as tile
    from concourse.bass_types import AP

    f32 = mybir.dt.float32
    f16 = mybir.dt.float16
    i16 = mybir.dt.int16
    i8 = mybir.dt.int8
    op = mybir.AluOpType
    Copy = mybir.ActivationFunctionType.Copy
    assert S % cs == 0
    G = S // cs

    nc = bacc.Bacc("TRN2", target_bir_lowering=False, debug=False,
                   num_devices=NCORES, num_swdge_queues=NQ)
    fm_t = nc.dram_tensor("fm", [FMI_ROWS, 2 * C], i8, kind="ExternalInput")
    idx_t = nc.dram_tensor("idx", [128, S * 8], i16, kind="ExternalInput")
    wgt_t = nc.dram_tensor("wgt", [128, 4 * S], f32, kind="ExternalInput")
    out_t = nc.dram_tensor("out", [128, S * C], f16, kind="ExternalOutput")

    # fm viewed as overlapping [entry, 4*C] rows with stride 2*C: one gathered
    # element covers interleaved entries (h,w) and (h,w+1) = the 2x2 footprint.
    fm_gather_ap = AP(fm_t, 0, [[2 * C, FMI_ROWS - 1], [1, 4 * C]])

    with tile.TileContext(nc) as tc:
        with (
            tc.tile_pool(name="const", bufs=1) as cpool,
            tc.tile_pool(name="gath", bufs=1) as gpool,
            tc.tile_pool(name="work", bufs=3) as wpool,
            tc.tile_pool(name="outp", bufs=2) as opool,
        ):
            def body():
                # per-group idx tiles so each gather waits only on its slice
                idxs = []
                for g in range(G):
                    ix = cpool.tile([128, cs * 8], i16, tag=f"idx{g}")
                    nc.sync.dma_start(ix[:], idx_t[:, g * cs * 8:(g + 1) * cs * 8])
                    idxs.append(ix)
                wgt = cpool.tile([128, 4 * S], f32, tag="wgt")
                nc.sync.dma_start(wgt[:], wgt_t[:])

                # all gathers up front: descgen on 4 Q7 pairs before DVE
                # starts locking the shared SBUF port
                gts = []
                for g in range(G):
                    gt = gpool.tile([128, cs, 4 * C], i8, tag=f"gt{g}")
                    nc.gpsimd.dma_gather(
                        gt[:], fm_gather_ap, idxs[g][:],
                        cs * 128, cs * 128, 4 * C, elem_step=2 * C,
                        queue_num=g % NQ,
                    )
                    gts.append(gt)

                for g in range(G):
                    gt = gts[g]
                    og = opool.tile([128, cs * C], f16, tag="og")
                    for sl in range(cs):
                        s = g * cs + sl
                        gv = gt[:, sl, :].rearrange(
                            "p (a b c) -> p a b c", a=2, b=2, c=C)
                        t1 = wpool.tile([128, 2, C], f16, tag="t1")
                        nc.scalar.activation(
                            t1[:], gv[:, :, 0, :], Copy,
                            bias=0.0, scale=wgt[:, 2 * S + s : 2 * S + s + 1])
                        st = wpool.tile([128, 2, C], f16, tag="st")
                        nc.vector.scalar_tensor_tensor(
                            st[:], gv[:, :, 1, :], wgt[:, s : s + 1], t1[:],
                            op.mult, op.add)
                        a2 = wpool.tile([128, C], f16, tag="a2")
                        if lerp == "dve":
                            nc.vector.tensor_scalar(
                                a2[:], st[:, 0, :],
                                wgt[:, 3 * S + s : 3 * S + s + 1], None, op.mult)
                        else:
                            nc.scalar.activation(
                                a2[:], st[:, 0, :], Copy, bias=0.0,
                                scale=wgt[:, 3 * S + s : 3 * S + s + 1])
                        nc.vector.scalar_tensor_tensor(
                            og[:, sl * C : (sl + 1) * C], st[:, 1, :],
                            wgt[:, S + s : S + s + 1], a2[:], op.mult, op.add)
                    nc.sync.dma_start(
                        out_t[:, g * cs * C : (g + 1) * cs * C], og[:])

            if reps == 1:
                body()
            else:
                with tc.For_i(0, reps, 1):
                    body()

    nc.compile()
    return nc


def _get_program(S: int):
    key = (S, 1, CS)
    if key not in _PROGRAM_CACHE:
        _PROGRAM_CACHE[key] = _build_program(S)
    return _PROGRAM_CACHE[key]


def _point_coords(r, off):
    """Per-point gather index + blend weights, float32 math matching the
    reference's edge rules.  r: [nb,5] rois, off: [nb, 2*NUM_POINT].
    Returns idx int32, lh, lw, ch float32, each [nb*P]."""
    r = r.astype(np.float32)
    off = off.astype(np.float32).reshape(-1, NUM_POINT, 2)
    cx = (r[:, 1] + r[:, 3]) * np.float32(0.5)
    cy = (r[:, 2] + r[:, 4]) * np.float32(0.5)
    w_ = r[:, 3] - r[:, 1] + np.float32(1.0)
    h_ = r[:, 4] - r[:, 2] + np.float32(1.0)
    ix = (cx[:, None] + off[:, :, 0] * w_[:, None] * np.float32(0.1)) / np.float32(STRIDE)
    iy = (cy[:, None] + off[:, :, 1] * h_[:, None] * np.float32(0.1)) / np.float32(STRIDE)
    wl = np.clip(np.floor(ix), 0, W - 1)
    lw = np.where(wl >= W - 1, np.float32(0.0), (ix - wl).astype(np.float32))
    hl = np.clip(np.floor(iy), 0, H - 1)
    lh = np.where(hl >= H - 1, np.float32(0.0), (iy - hl).astype(np.float32))
    idx = (hl * W + wl).astype(np.int32).reshape(-1)
    return idx, lh.reshape(-1).astype(np.float32), lw.reshape(-1).astype(np.float32), (
        np.float32(1.0) - lh.reshape(-1)
    ).astype(np.float32)


def _host_prep(feat_map, rois, offset, num_point):
    """Route rois by batch index; build per-core fm/idx/weight inputs."""
    bidx = rois[:, 0].astype(np.int32)
    ids = [np.nonzero(bidx == b)[0] for b in range(B)]
    cap = max(len(i) for i in ids)
    S = math.ceil(max(cap * num_point, 1) / 128)
    S = ((S + CS - 1) // CS) * CS
    NP = S * 128

    in_maps = []
    perms = []
    for b in range(B):
        fmhwc = np.ascontiguousarray(feat_map[b].transpose(1, 2, 0))  # [H,W,C]
        delta = np.float32(np.abs(fmhwc).max() / 127.0) or np.float32(1.0)
        q = np.clip(np.rint(fmhwc * (np.float32(1.0) / delta)), -127, 127).astype(np.int8)
        fmi = np.zeros((FMI_ROWS, 2 * C), np.int8)
        fmi[: H * W, :C] = q.reshape(H * W, C)
        fmi[: (H - 1) * W, C:] = q[1:].reshape((H - 1) * W, C)
        idxf = np.zeros(NP, np.int16)  # padding gathers entry 0, weights 0
        w4 = np.zeros((4, NP), np.float32)  # lh*delta, lw, ch*delta, cw
        idl = ids[b]
        nb = len(idl)
        perm = None
        if nb:
            pidx, lh, lw, ch = _point_coords(rois[idl], offset[idl])
            npts = nb * num_point
            perm = np.argsort(pidx, kind="stable")  # HBM locality for gather
            idxf[:npts] = pidx[perm].astype(np.int16)
            w4[0, :npts] = (lh * delta)[perm]
            w4[1, :npts] = lw[perm]
            w4[2, :npts] = (ch * delta)[perm]
            w4[3, :npts] = (np.float32(1.0) - lw)[perm]
        perms.append(perm)
        # gather-order layout: point q=(s*128+p) -> idx col s*8 + p//16,
        # partition p%16, replicated to all 128 partitions.
        idx16 = np.ascontiguousarray(idxf.reshape(S * 8, 16).T)  # [16, S*8]
        wgt = np.concatenate(
            [a.reshape(S, 128).T for a in w4], axis=1).astype(np.float32)
        in_maps.append({"fm": fmi, "idx": np.tile(idx16, (8, 1)),
                        "wgt": np.ascontiguousarray(wgt)})
    return ids, perms, S, in_maps


def _host_unshard(results, ids, perms, S, num_point, n):
    out_full = np.zeros((n, num_point, C), np.float32)
    for b in range(B):
        nb = len(ids[b])
        if not nb:
            continue
        o = results[b]["out"].reshape(128, S, C).transpose(1, 0, 2).reshape(S * 128, C)
        npts = nb * num_point
        unsorted = np.empty((npts, C), np.float32)
        unsorted[perms[b]] = o[:npts].astype(np.float32)
        out_full[ids[b]] = unsorted.reshape(nb, num_point, C)
    return out_full


def kernel(feat_map, rois, offset, stride, num_point, _collect=None):
    from concourse.bass_utils import run_bass_kernel_spmd

    feat_map = np.ascontiguousarray(np.asarray(feat_map, np.float32))
    rois = np.asarray(rois, np.float32)
    offset = np.asarray(offset, np.float32)
    stride = int(stride)
    num_point = int(num_point)
    assert feat_map.shape == (B, C, H, W), feat_map.shape
    assert stride == STRIDE and num_point == NUM_POINT

    ids, perms, S, in_maps = _host_prep(feat_map, rois, offset, num_point)
    nc = _get_program(S)
    res = run_bass_kernel_spmd(nc, in_maps, core_ids=list(range(NCORES)),
                               **(_collect.pop("spmd_kwargs", {}) if _collect else {}))
    if _collect is not None:
        _collect["res"] = res
    return _host_unshard(res.results, ids, perms, S, num_point, rois.shape[0])
